# revision 19
# baseline (speedup 1.0000x reference)
"""2-layer GCN on 8 Trainium2 NeuronCores — aggregate-then-project.

Nodes are range-sharded across 8 cores (dst parallel). Both GCN layers are
computed as: gather source rows from a bf16 feature table (dma_gather with
biased signed-int16 indices), segment-sum via one-hot bf16 matmuls into PSUM,
then per-dst-tile projection:

  layer1 table = dinv*x (host-precomputed, full -> no collective needed)
      aggx[d]  = sum_{s->d} table1[s]          (self-loops applied densely)
      t2[d]    = dinv[d] * relu(dinv[d]*(aggx[d] @ W1) + b1)
  AllGather t2 -> table2  (pair-packed: row r = [t2[2r-tile], t2[2r+1-tile]],
      so the collective moves only the real 64 features; 4 chunked AllGathers
      overlap the tail of layer 1)
      out[d]   = dinv[d]*(agg2[d] @ W2) + b2

Layer-2 gather descriptors for the first P_GROUPS groups are pre-generated
with prepare_only during the AllGather window and fired by trigger_dma when
table2 lands, hiding the SWDGE descriptor-generation ramp.
"""
import os
import sys

sys.path.insert(0, "/opt/trn_rl_repo")

import numpy as np
import ml_dtypes

import concourse.bass as bass
import concourse.bacc as bacc
import concourse.tile as tile
import concourse.mybir as mybir
from concourse import bass_utils
from concourse.library_config import mlp

N_CORES = 8
N_NODES = 100000
D_IN, D_H, D_OUT = 128, 64, 64
NSHARD = N_NODES // N_CORES          # 12500
TILE = 128
NT = (NSHARD + TILE - 1) // TILE     # 98
PADN = NT * TILE                     # 12544
PADN_ALL = N_CORES * PADN            # 100352
PROWS = PADN // 2                    # 6272  (pair-packed rows per core)
PROWS_ALL = N_CORES * PROWS          # 50176
B0_ROWS = 50176                      # layer-1 bucket split (int16 bias)
BIAS1 = (32768, B0_ROWS + 32768)
BIAS2 = PROWS_ALL // 2               # 25088 (single bucket for layer 2)
N_BUCKET = 2
GROUP = 4
GROUPS = [(t, min(t + GROUP, NT)) for t in range(0, NT, GROUP)]
N_GROUP = len(GROUPS)                # 25 (24x4 tiles + 1x2)

P_GROUPS = 0                         # prepare_only path delivers no data on
                                     # this runtime -- keep disabled
AG_AFTER = (N_GROUP - 1,)            # AllGather boundaries (group idx)
# packed-row bounds per AllGather chunk; t2_full is chunk-major so each
# collective writes one contiguous [8*chunk_rows, 128] block
AG_BOUNDS = [0] + [(GROUPS[g][1] + 1) // 2 * 128 for g in AG_AFTER]
SHARED_T2 = True                     # single AllGather writer -> Shared OK


LAST_RESULT = None


def _pack_meta(core, t_id, dloc, idx_val_raw, bucket):
    """Slot layout + per-core idx16/dstloc for one edge pass.

    core/t_id/dloc: destination core, dst tile, dst row within tile.
    idx_val_raw: biased int16-range gather index per edge.
    bucket: 0/1 sub-bucket per edge (layer 1: table half for bias;
            layer 2: src-tile parity selecting the gathered column half).
    """
    key = (core * NT + t_id) * N_BUCKET + bucket
    order = np.argsort(key, kind="stable")
    key_s = key[order]
    idx_s = idx_val_raw[order]
    dloc_s = dloc[order]

    ngroups = N_CORES * NT * N_BUCKET
    counts = np.bincount(key_s, minlength=ngroups).reshape(N_CORES, NT, N_BUCKET)
    nb = -(-counts.max(axis=0) // 128)                  # [NT, N_BUCKET] ceil
    nb = np.maximum(nb, 1)
    # each sub-gather ends at (tmid-1, b) or (t1-1, b); those (t,b) must end
    # with >=1 pad slot on every core (the gather ucode trims trailing
    # negative idxs, which would otherwise drop real edges).
    for (t0, t1) in GROUPS:
        tmid = (t0 + t1) // 2
        for tf in (tmid - 1, t1 - 1):
            for b in range(N_BUCKET):
                if (counts[:, tf, b] == nb[tf, b] * 128).any():
                    nb[tf, b] += 1

    # slot layout: per group of GROUP tiles: all b0 chunks (tile-major), then
    # all b1 chunks -> one contiguous gather dst region per (group, bucket).
    chunk_col = np.zeros((NT, N_BUCKET), np.int64)
    grp_nc = np.zeros(N_GROUP, np.int64)
    grp_base = np.zeros(N_GROUP, np.int64)
    grp_b_off = np.zeros((N_GROUP, N_BUCKET + 1), np.int64)
    pos = 0
    for g, (t0, t1) in enumerate(GROUPS):
        grp_base[g] = pos
        for b in range(N_BUCKET):
            grp_b_off[g, b] = pos - grp_base[g]
            for t in range(t0, t1):
                chunk_col[t, b] = pos
                pos += nb[t, b]
        grp_nc[g] = pos - grp_base[g]
        grp_b_off[g, N_BUCKET] = grp_nc[g]
    CHC = pos
    IDXC16 = CHC * 8

    grp_start = np.zeros(ngroups + 1, np.int64)
    np.cumsum(counts.reshape(-1), out=grp_start[1:])
    rank = np.arange(key_s.shape[0], dtype=np.int64) - grp_start[key_s]

    core_s = key_s // (NT * N_BUCKET)
    tb = key_s % (NT * N_BUCKET)
    t_s = tb // N_BUCKET
    b_s = tb % N_BUCKET

    slot = chunk_col[t_s, b_s] * 128 + rank
    ccol = slot // 128
    cpart = slot % 128

    assert idx_s.min() >= -32768 and idx_s.max() <= 32767

    idx16_16 = np.zeros((N_CORES, 16, IDXC16), np.int16)
    idx16_16[core_s, slot % 16, slot // 16] = idx_s.astype(np.int16)
    idx16 = np.tile(idx16_16, (1, 8, 1))                # [cores, 128, IDXC16]

    dstloc = np.full((N_CORES, 128, CHC), 512.0, np.float32)
    dstloc[core_s, cpart, ccol] = dloc_s.astype(np.float32)
    dstloc = dstloc.astype(ml_dtypes.bfloat16)

    return dict(nb=nb, chunk_col=chunk_col, grp_nc=grp_nc, grp_base=grp_base,
                grp_b_off=grp_b_off, CHC=CHC, IDXC16=IDXC16,
                idx16=idx16, dstloc=dstloc)


def _host_prep(x, edge_index):
    src = np.asarray(edge_index[0], dtype=np.int64)
    dst = np.asarray(edge_index[1], dtype=np.int64)
    n = N_NODES

    deg = np.bincount(dst, minlength=n).astype(np.float64) + 1.0
    dinv = (1.0 / np.sqrt(deg)).astype(np.float32)

    core_d = dst // NSHARD
    drem = dst % NSHARD
    t_id = drem // TILE
    dloc = drem % TILE

    # layer 1: table = dinv*x padded to [PADN_ALL, 128]
    gsrc1 = (src // NSHARD) * PADN + (src % NSHARD)
    b1 = (gsrc1 >= B0_ROWS).astype(np.int64)
    idxv1 = gsrc1 - np.where(b1 == 0, BIAS1[0], BIAS1[1])
    meta1 = _pack_meta(core_d, t_id, dloc, idxv1, b1)

    # layer 2: pair-packed t2 table [PROWS_ALL, 128]; node (core,tile,p) has
    # local packed row j = (tile//2)*128 + p, column half = tile%2. t2_full is
    # chunk-major (AllGather chunk c: all 8 cores' rows [r0_c, r1_c)
    # contiguously), so prow = 8*r0_c + core*(r1_c-r0_c) + (j - r0_c).
    s_core = src // NSHARD
    s_loc = src % NSHARD
    s_tile = s_loc // TILE
    s_p = s_loc % TILE
    j = (s_tile // 2) * 128 + s_p
    bounds = np.asarray(AG_BOUNDS, np.int64)
    c = np.searchsorted(bounds[1:], j, side="right")
    r0c = bounds[c]
    szc = bounds[c + 1] - r0c
    prow = 8 * r0c + s_core * szc + (j - r0c)
    half = s_tile % 2
    idxv2 = prow - BIAS2
    meta2 = _pack_meta(core_d, t_id, dloc, idxv2, half)

    dinv_cols = np.zeros((N_CORES, 128, NT), np.float32)
    node_grid = (
        np.arange(N_CORES)[:, None, None] * NSHARD
        + np.arange(NT)[None, None, :] * TILE
        + np.arange(128)[None, :, None]
    )
    local = np.arange(NT)[None, None, :] * TILE + np.arange(128)[None, :, None]
    valid = np.broadcast_to(local < NSHARD, node_grid.shape)
    dinv_cols[:] = np.where(valid, dinv[np.where(valid, node_grid, 0)], 0.0)

    xt = np.zeros((PADN_ALL, D_IN), np.float32)
    xs = (np.asarray(x, np.float32) * dinv[:, None]).reshape(N_CORES, NSHARD, D_IN)
    xt.reshape(N_CORES, PADN, D_IN)[:, :NSHARD] = xs
    xt = xt.astype(ml_dtypes.bfloat16)

    # per-core dinv*x re-tiled so a group's 4 dst tiles load as one
    # contiguous-per-partition DMA: xg[k, p, t*128+f] = xt[k, t*128+p, f]
    xg = np.ascontiguousarray(
        xt.reshape(N_CORES, NT, 128, D_IN).transpose(0, 2, 1, 3)
        .reshape(N_CORES, 128, NT * D_IN))

    return xt, xg, dinv_cols, meta1, meta2


def _build_program(meta1, meta2, b1_zero, b2_zero):
    f32 = mybir.dt.float32
    bf16 = mybir.dt.bfloat16
    i16 = mybir.dt.int16
    nc = bacc.Bacc("TRN2", target_bir_lowering=False, debug=False,
                   num_devices=N_CORES, num_swdge_queues=4)

    xt_in = nc.dram_tensor("xt", [PADN_ALL, D_IN], bf16, kind="ExternalInput").ap()
    xg_in = nc.dram_tensor("xg", [128, NT * D_IN], bf16, kind="ExternalInput").ap()
    idb_in = nc.dram_tensor("identb", [128, 128], bf16, kind="ExternalInput").ap()
    w1_in = nc.dram_tensor("W1b", [D_IN, D_H], bf16, kind="ExternalInput").ap()
    w2_in = nc.dram_tensor("W2b", [D_H, D_OUT], bf16, kind="ExternalInput").ap()
    b1_in = nc.dram_tensor("b1r", [128, D_H], f32, kind="ExternalInput").ap()
    b2_in = nc.dram_tensor("b2r", [128, D_OUT], f32, kind="ExternalInput").ap()
    io_in = nc.dram_tensor("iota", [128, 128], bf16, kind="ExternalInput").ap()
    dv_in = nc.dram_tensor("dinv_cols", [128, NT], f32, kind="ExternalInput").ap()
    ix1_in = nc.dram_tensor("idx16_1", [128, meta1["IDXC16"]], i16,
                            kind="ExternalInput").ap()
    dl1_in = nc.dram_tensor("dstloc_1", [128, meta1["CHC"]], bf16,
                            kind="ExternalInput").ap()
    ix2_in = nc.dram_tensor("idx16_2", [128, meta2["IDXC16"]], i16,
                            kind="ExternalInput").ap()
    dl2_in = nc.dram_tensor("dstloc_2", [128, meta2["CHC"]], bf16,
                            kind="ExternalInput").ap()
    out_t = nc.dram_tensor("out", [PADN, D_OUT], bf16, kind="ExternalOutput").ap()

    rg = [list(range(N_CORES))]

    with tile.TileContext(nc) as tc:
        with tc.tile_pool(name="const", bufs=1) as constp, \
             tc.tile_pool(name="dram", bufs=1, space="DRAM") as dram, \
             tc.tile_pool(name="agg", bufs=3, space="PSUM") as aggp, \
             tc.tile_pool(name="proj", bufs=2, space="PSUM") as projp, \
             tc.tile_pool(name="tp", bufs=2, space="PSUM") as tpp, \
             tc.tile_pool(name="gat", bufs=4) as gatp, \
             tc.tile_pool(name="sel", bufs=3) as selp, \
             tc.tile_pool(name="dx", bufs=3) as dxp, \
             tc.tile_pool(name="og", bufs=2) as ogp, \
             tc.tile_pool(name="sb", bufs=3) as sb:

            nc.gpsimd.load_library(mlp)

            w1 = constp.tile([D_IN, D_H], bf16)
            nc.sync.dma_start(w1[:], w1_in[:])
            w2 = constp.tile([D_H, D_OUT], bf16)
            nc.sync.dma_start(w2[:], w2_in[:])
            b1r = constp.tile([128, D_H], f32)
            nc.sync.dma_start(b1r[:], b1_in[:])
            b2r = constp.tile([128, D_OUT], f32)
            nc.sync.dma_start(b2r[:], b2_in[:])
            iota = constp.tile([128, 128], bf16)
            nc.sync.dma_start(iota[:], io_in[:])
            identb = constp.tile([128, 128], bf16)
            nc.sync.dma_start(identb[:], idb_in[:])
            dvc = constp.tile([128, NT], f32)
            nc.sync.dma_start(dvc[:], dv_in[:])

            # layer-1 idx loads in two pieces so the first gathers only wait
            # on the small head chunk; layer-2 metadata loads behind them.
            IDXC1, CHC1 = meta1["IDXC16"], meta1["CHC"]
            IDXC2, CHC2 = meta2["IDXC16"], meta2["CHC"]
            ix1 = constp.tile([128, IDXC1], i16)
            ix_head = int(meta1["grp_base"][2]) * 8 if N_GROUP > 2 else IDXC1
            nc.sync.dma_start(ix1[:, 0:ix_head], ix1_in[:, 0:ix_head])
            nc.sync.dma_start(ix1[:, ix_head:IDXC1], ix1_in[:, ix_head:IDXC1])
            dl1 = constp.tile([128, CHC1], bf16)
            nc.sync.dma_start(dl1[:], dl1_in[:])
            ix2 = constp.tile([128, IDXC2], i16)
            nc.sync.dma_start(ix2[:], ix2_in[:])
            dl2 = constp.tile([128, CHC2], bf16)
            nc.sync.dma_start(dl2[:], dl2_in[:])

            # all of t2 stays resident in SBUF for layer-2 self-loops
            t2sb = constp.tile([128, NT, D_H], bf16)

            t2_shard = dram.tile([PROWS, 128], bf16)
            if SHARED_T2:
                t2_full = dram.tile([PROWS_ALL, 128], bf16, addr_space="Shared")
            else:
                t2_full = dram.tile([PROWS_ALL, 128], bf16)

            last_gather = [None] * 4      # most recent gather per queue
            first_after_trigger = [None] * 4

            def issue_gathers(g, t0g, t1g, M, tbs, ixall, prep=False,
                              sems=None, extra_deps=()):
                base = int(M["grp_base"][g])
                ncg = int(M["grp_nc"][g])
                chunk_col = M["chunk_col"]
                grp_b_off = M["grp_b_off"]
                G = gatp.tile([128, ncg, D_IN], bf16, tag="G")
                # 4 sub-gathers per group (one per SWDGE queue): each
                # bucket's chunk range split at a tile boundary so every
                # sub-gather still ends in pad slots (trailing-trim safe).
                qn = g
                for b in range(N_BUCKET):
                    tmid = (t0g + t1g) // 2
                    lo = int(grp_b_off[g, b])
                    mid = int(chunk_col[tmid, b]) - base
                    hi = int(grp_b_off[g, b + 1])
                    for c0, c1 in ((lo, mid), (mid, hi)):
                        ncb = c1 - c0
                        if ncb == 0:
                            continue
                        nidx = ncb * 128
                        q = qn % 4
                        kw = {}
                        if prep:
                            kw = dict(prepare_only=True, sem=sems[q])
                        bi = nc.gpsimd.dma_gather(
                            G[:, c0:c1, :],
                            tbs[b],
                            ixall[:, (base + c0) * 8:(base + c1) * 8],
                            nidx, nidx, D_IN,
                            single_packet=False,
                            queue_num=q,
                            **kw,
                        )
                        if prep and last_gather[q] is not None:
                            # keep ring order: preps must enter the queue
                            # after every normal layer-1 gather on it
                            d = bass.InstructionNameOrderedSet()
                            d.add(last_gather[q].ins.name)
                            bi.ins.add_nosync_dependencies_from(d)
                        if (not prep) and first_after_trigger[q] is not None:
                            d = bass.InstructionNameOrderedSet()
                            d.add(first_after_trigger[q].ins.name)
                            bi.ins.add_nosync_dependencies_from(d)
                            first_after_trigger[q] = None
                        for dep in extra_deps:
                            # table chunks outside the biased AP slice
                            tile.add_dep_helper(
                                bi.ins, dep.ins,
                                reason="AllGather chunk feeds gather table")
                        last_gather[q] = bi
                        qn += 1
                return G

            def consume_group(g, t0g, t1g, G, M, dl, layer):
                base = int(M["grp_base"][g])
                nb = M["nb"]
                chunk_col = M["chunk_col"]
                width = D_IN if layer == 1 else D_H
                ntl = t1g - t0g
                if layer == 1:
                    dxt_g = dxp.tile([128, GROUP * D_IN], bf16, tag="dx")
                    nc.sync.dma_start(
                        dxt_g[:, 0:ntl * D_IN],
                        xg_in[:, t0g * D_IN:t1g * D_IN])
                else:
                    og = ogp.tile([128, GROUP, D_OUT], bf16, tag="og")
                for t in range(t0g, t1g):
                    ti = t - t0g
                    nb0 = int(nb[t, 0])
                    nb1 = int(nb[t, 1])
                    nct = nb0 + nb1
                    l0 = int(chunk_col[t, 0]) - base
                    l1 = int(chunk_col[t, 1]) - base
                    S0 = selp.tile([128, nb0, 128], bf16, tag="S0")
                    nc.vector.tensor_tensor(
                        out=S0[:],
                        in0=dl[:, base + l0:base + l0 + nb0]
                            .to_broadcast([128, nb0, 128]),
                        in1=iota[:].unsqueeze(1).to_broadcast([128, nb0, 128]),
                        op=mybir.AluOpType.is_equal,
                    )
                    S1 = selp.tile([128, nb1, 128], bf16, tag="S1")
                    nc.vector.tensor_tensor(
                        out=S1[:],
                        in0=dl[:, base + l1:base + l1 + nb1]
                            .to_broadcast([128, nb1, 128]),
                        in1=iota[:].unsqueeze(1).to_broadcast([128, nb1, 128]),
                        op=mybir.AluOpType.is_equal,
                    )
                    if layer == 1:
                        # aggT[f, d] = sum_e G[e, f] * S[e, d]  (G stationary,
                        # S moving) -> agg arrives pre-transposed for the
                        # feature-contraction projection matmul.
                        aggT = aggp.tile([D_IN, 128], f32, tag="agg")
                        dxt = dxt_g[:, ti * D_IN:(ti + 1) * D_IN]
                        # self-loop: aggT += dxt.T @ I (dense local rows)
                        nc.tensor.matmul(aggT[:], lhsT=dxt, rhs=identb[:],
                                         start=True, stop=False)
                        for i in range(nct):
                            S = S0[:, i, :] if i < nb0 else S1[:, i - nb0, :]
                            gc = (l0 + i) if i < nb0 else (l1 + i - nb0)
                            nc.tensor.matmul(
                                aggT[:], lhsT=G[:, gc, :], rhs=S,
                                start=False, stop=(i == nct - 1),
                            )
                        aggT_sb = sb.tile([D_IN, 128], bf16, tag="e1")
                        nc.scalar.copy(aggT_sb[:], aggT[:])
                        # t2 = dinv*relu(dinv*(agg @ W1) + b1)
                        proj = projp.tile([128, D_H], f32, tag="proj")
                        nc.tensor.matmul(proj[:], lhsT=aggT_sb[:], rhs=w1[:],
                                         start=True, stop=True)
                        if b1_zero:
                            hr = sb.tile([128, D_H], f32, tag="e5")
                            nc.scalar.activation(
                                hr[:], proj[:],
                                mybir.ActivationFunctionType.Relu,
                                scale=dvc[:, t:t + 1])
                        else:
                            hv = sb.tile([128, D_H], f32, tag="e3")
                            nc.scalar.activation(
                                hv[:], proj[:],
                                mybir.ActivationFunctionType.Copy,
                                scale=dvc[:, t:t + 1])
                            hb = sb.tile([128, D_H], f32, tag="e4")
                            nc.vector.tensor_add(hb[:], hv[:], b1r[:])
                            hr = sb.tile([128, D_H], f32, tag="e5")
                            nc.scalar.activation(
                                hr[:], hb[:],
                                mybir.ActivationFunctionType.Relu)
                        t2t = sb.tile([128, D_H], bf16, tag="e6")
                        nc.scalar.activation(
                            t2t[:], hr[:],
                            mybir.ActivationFunctionType.Copy,
                            scale=dvc[:, t:t + 1])
                        # pre-project the layer-2 table by W2 (linearity of
                        # the aggregation): table2 = t2 @ W2, so layer 2 is
                        # pure aggregation with no per-tile projection.
                        t2T_ps = tpp.tile([D_H, 128], f32, tag="tp")
                        nc.tensor.matmul(t2T_ps[:], lhsT=t2t[:],
                                         rhs=identb[:], start=True, stop=True)
                        t2T_sb = sb.tile([D_H, 128], bf16, tag="e7")
                        nc.scalar.copy(t2T_sb[:], t2T_ps[:])
                        t2p_ps = projp.tile([128, D_OUT], f32, tag="proj")
                        nc.tensor.matmul(t2p_ps[:], lhsT=t2T_sb[:], rhs=w2[:],
                                         start=True, stop=True)
                        nc.scalar.copy(t2sb[:, t, :], t2p_ps[:])
                    else:
                        # pure aggregation: agg[d, o] = sum_e S[e, d]*Gp[e, o]
                        # (S stationary 128-wide -> fast LDW; G moving 64).
                        agg = aggp.tile([128, D_OUT], f32, tag="agg")
                        nc.tensor.matmul(agg[:], lhsT=identb[:],
                                         rhs=t2sb[:, t, :],
                                         start=True, stop=False)
                        for i in range(nct):
                            S = S0[:, i, :] if i < nb0 else S1[:, i - nb0, :]
                            gc = (l0 + i) if i < nb0 else (l1 + i - nb0)
                            h = 0 if i < nb0 else 1
                            nc.tensor.matmul(
                                agg[:], lhsT=S,
                                rhs=G[:, gc, h * D_H:(h + 1) * D_H],
                                start=False, stop=(i == nct - 1),
                            )
                        # out = dinv*agg + b2
                        if b2_zero:
                            nc.scalar.activation(
                                og[:, ti, :], agg[:],
                                mybir.ActivationFunctionType.Copy,
                                scale=dvc[:, t:t + 1])
                        else:
                            ov = sb.tile([128, D_OUT], f32, tag="e3")
                            nc.scalar.activation(
                                ov[:], agg[:],
                                mybir.ActivationFunctionType.Copy,
                                scale=dvc[:, t:t + 1])
                            nc.vector.tensor_add(og[:, ti, :], ov[:], b2r[:])
                if layer == 1:
                    # pair-packed t2 write: tiles (2u, 2u+1) -> packed rows
                    # u*128..u*128+127, column halves 0/1. One DMA per
                    # row-block of two tiles.
                    for j in range((ntl + 1) // 2):
                        u = t0g // 2 + j
                        tl = t0g + 2 * j
                        nhalf = min(2, t1g - tl)
                        nc.sync.dma_start(
                            t2_shard[u * 128:(u + 1) * 128,
                                     0:nhalf * D_H],
                            t2sb[:, tl:tl + nhalf, :])
                else:
                    nc.sync.dma_start(
                        out_t[t0g * 128:t1g * 128, :]
                        .rearrange("(i p) f -> p i f", i=ntl),
                        og[:, 0:ntl, :])

            tb1 = (xt_in[BIAS1[0]:B0_ROWS, :], xt_in[BIAS1[1]:PADN_ALL, :])
            tb2s = t2_full[BIAS2:PROWS_ALL, :]
            tb2 = (tb2s, tb2s)

            # ---- layer 1 + chunked AllGather ----
            ag_insts = []
            prev_rows = 0
            for gp in range(N_GROUP):
                ga = issue_gathers(gp, *GROUPS[gp], meta1, tb1, ix1)
                consume_group(gp, *GROUPS[gp], ga, meta1, dl1, layer=1)
                if gp in AG_AFTER:
                    r1 = (GROUPS[gp][1] + 1) // 2 * 128
                    ag = nc.gpsimd.collective_compute(
                        "AllGather", mybir.AluOpType.bypass,
                        ins=[t2_shard[prev_rows:r1, :]],
                        outs=[t2_full[8 * prev_rows:
                                      8 * prev_rows + 8 * (r1 - prev_rows), :]],
                        replica_groups=rg,
                    )
                    ag_insts.append(ag)
                    prev_rows = r1

            # ---- layer-2 descriptor prep during the AllGather window ----
            prep_G = []
            if P_GROUPS > 0:
                prep_sems = [nc.alloc_semaphore(f"l2prep_q{q}")
                             for q in range(4)]
                for gp in range(P_GROUPS):
                    ga = issue_gathers(gp, *GROUPS[gp], meta2, tb2, ix2,
                                       prep=True, sems=prep_sems)
                    prep_G.append(ga)
                for q in range(4):
                    trig = nc.gpsimd.trigger_dma(count=None, queue_num=q)
                    for dep in ag_insts:
                        tile.add_dep_helper(
                            trig.ins, dep.ins,
                            reason="AllGather chunk feeds gather table")
                    first_after_trigger[q] = trig

            # ---- layer 2 ----
            for gp in range(N_GROUP):
                if gp < P_GROUPS:
                    ga = prep_G[gp]
                else:
                    ga = issue_gathers(gp, *GROUPS[gp], meta2, tb2, ix2,
                                       extra_deps=ag_insts)
                consume_group(gp, *GROUPS[gp], ga, meta2, dl2, layer=2)

    nc.compile()
    return nc


def kernel(x, edge_index, W1, b1, W2, b2):
    global LAST_RESULT
    x = np.asarray(x, np.float32)
    W1 = np.asarray(W1, np.float32)
    W2 = np.asarray(W2, np.float32)
    b1 = np.asarray(b1, np.float32)
    b2 = np.asarray(b2, np.float32)

    xt, xg, dinv_cols, meta1, meta2 = _host_prep(x, edge_index)
    nc = _build_program(meta1, meta2, bool(np.all(b1 == 0.0)),
                        bool(np.all(b2 == 0.0)))

    iota = np.tile(np.arange(128, dtype=np.float32), (128, 1)).astype(
        ml_dtypes.bfloat16)
    identb = np.eye(128, dtype=np.float32).astype(ml_dtypes.bfloat16)
    b1r = np.tile(b1[None, :], (128, 1)).astype(np.float32)
    b2r = np.tile(b2[None, :], (128, 1)).astype(np.float32)
    w1b = W1.astype(ml_dtypes.bfloat16)
    w2b = W2.astype(ml_dtypes.bfloat16)

    in_maps = []
    for k in range(N_CORES):
        in_maps.append({
            "xt": xt, "xg": xg[k], "identb": identb,
            "W1b": w1b, "W2b": w2b, "b1r": b1r, "b2r": b2r,
            "iota": iota,
            "dinv_cols": dinv_cols[k],
            "idx16_1": meta1["idx16"][k],
            "dstloc_1": meta1["dstloc"][k],
            "idx16_2": meta2["idx16"][k],
            "dstloc_2": meta2["dstloc"][k],
        })

    trace = bool(os.environ.get("BASS_TRACE"))
    res = bass_utils.run_bass_kernel_spmd(
        nc, in_maps, core_ids=list(range(N_CORES)), trace=trace)
    LAST_RESULT = res

    out = np.empty((N_NODES, D_OUT), np.float32)
    for k in range(N_CORES):
        out[k * NSHARD:(k + 1) * NSHARD] = np.asarray(
            res.results[k]["out"][:NSHARD], dtype=np.float32)
    return out
